# revision 14
# baseline (speedup 1.0000x reference)
"""Trainium2 Bass kernel for a pre-norm transformer block (dense_transformer).

Shapes (hardcoded): x [B=4, N=2048, C=384], HEADS=6, HEAD_DIM=64, HID=1536.

Sharding: 8 cores = (batch, query-half). Core c handles batch b=c//2 and query
rows half=c%2. Each core receives its batch's full 2048 tokens, reordered so
its own 1024 query rows come first (attention keys are permutation-invariant).
It computes LN1 -> QKV (K/V for all 2048 tokens, Q for its 1024), dense
attention for all 6 heads, proj + residual, LN2, MLP + residual, and writes its
1024 output rows. No cross-core communication.

Device-side layout choices:
  - Activations cast to bf16 for matmuls; residual stream stays fp32.
  - Feature-major ("transposed") activations hT=[C, T] produced via SBUF->SBUF
    DMA-xbar transposes of the LN output, so every matmul contracts over the
    partition dim.
  - Scores computed transposed per head: S^T[k, q] = K_h Q_h^T with two heads
    sharing the 128-row PE array via tile_position row packing (head_dim=64).
  - Softmax skips the max-subtraction (scores are provably < ~30 here, exp
    cannot overflow fp32) and gets the denominator for free from a ones
    column inside the padded V block. exp runs on ScalarE directly out of PSUM; the 1/sqrt(d)
    scale is folded into Wq/Wk host-side, LN gain/bias folded into all weights.
"""

import math

import numpy as np
import ml_dtypes

B, N, C = 4, 2048, 384
HEADS, HEAD_DIM = 6, 64
HID = 1536
EPS = 1e-5
NCORES = 8
T = N            # tokens per core (full batch element)
TQ = N // 2      # query rows per core
CC = C // 128    # 3 feature chunks
NT = T // 128    # 16 token chunks
NTQ = TQ // 128  # 8 query-token chunks
MH = HID // 128  # 12 hidden chunks

_COMPILED = None  # (nc, input_names) cache


def build_nc(sim_gelu=False):
    """Build + compile the per-core Bass/Tile program (same for all cores)."""
    import concourse.bass as bass
    import concourse.tile as tile
    from concourse import bacc, mybir

    f32 = mybir.dt.float32
    bf16 = mybir.dt.bfloat16
    AF = mybir.ActivationFunctionType
    ALU = mybir.AluOpType

    nc = bacc.Bacc("TRN2", target_bir_lowering=False, debug=False,
                   num_devices=NCORES)

    xkv_d = nc.dram_tensor("xkv", [T, C], f32, kind="ExternalInput").ap()
    wqk_d = nc.dram_tensor("wqk", [C, 2 * C], bf16, kind="ExternalInput").ap()
    bqk_d = nc.dram_tensor("bqk", [2 * C], f32, kind="ExternalInput").ap()
    wv_d = nc.dram_tensor("wv", [C, C], bf16, kind="ExternalInput").ap()
    bv_d = nc.dram_tensor("bv", [C], f32, kind="ExternalInput").ap()
    wp_d = nc.dram_tensor("wp", [C, C], bf16, kind="ExternalInput").ap()
    bp_d = nc.dram_tensor("bp", [C], f32, kind="ExternalInput").ap()
    w1_d = nc.dram_tensor("w1", [C, HID], bf16, kind="ExternalInput").ap()
    b1_d = nc.dram_tensor("b1", [HID], f32, kind="ExternalInput").ap()
    w2_d = nc.dram_tensor("w2", [HID, C], bf16, kind="ExternalInput").ap()
    b2_d = nc.dram_tensor("b2", [C], f32, kind="ExternalInput").ap()
    out_d = nc.dram_tensor("out", [TQ, C], f32, kind="ExternalOutput").ap()

    with tile.TileContext(nc) as tc:
        with (
            tc.tile_pool(name="singles", bufs=1) as singles,
            tc.tile_pool(name="work", bufs=3) as work,
            tc.tile_pool(name="stats", bufs=6) as stats,
            tc.tile_pool(name="attn", bufs=3) as attn_pool,
            tc.tile_pool(name="psumA", bufs=2, space="PSUM") as psumA,
            tc.tile_pool(name="psumB", bufs=2, space="PSUM") as psumB,
            tc.tile_pool(name="dram", bufs=2, space="DRAM") as dram,
        ):
            # ---- persistent SBUF tensors ----
            xq = singles.tile([128, NTQ, C], f32, tag="xq")       # q-half x rows
            hT = [singles.tile([128, T], bf16, tag=f"hT{c}", name=f"hT{c}") for c in range(CC)]
            qT = [singles.tile([128, TQ], bf16, tag=f"qT{c}", name=f"qT{c}") for c in range(CC)]
            kT = [singles.tile([128, T], bf16, tag=f"kT{c}", name=f"kT{c}") for c in range(CC)]
            # V padded to 128 cols per head so each head's PV output lands on
            # the oT partitions it belongs to. Even heads: [V(0:64)|ones(64)|0].
            # Odd heads: [ones(0)|0(1:64)|V(64:128)]. The ones column yields
            # the softmax denominator row right next to the head's O block.
            vaug = singles.tile([128, NT, HEADS, 128], bf16, tag="vaug")
            oT = [singles.tile([128, TQ], bf16, tag=f"oT{c}", name=f"oT{c}") for c in range(CC)]
            x2 = singles.tile([128, NTQ, C], f32, tag="x2")
            x2T = [singles.tile([128, TQ], bf16, tag=f"x2T{c}", name=f"x2T{c}") for c in range(CC)]
            gT = singles.tile([128, MH, TQ], bf16, tag="gT")
            eps_t = singles.tile([128, 1], f32, tag="eps")
            nc.vector.memset(eps_t, EPS)

            # ---- weights ----
            wqk = singles.tile([128, CC, 2 * C], bf16, tag="wqk")
            nc.sync.dma_start(wqk, wqk_d.rearrange("(c p) f -> p c f", p=128))
            bqk = singles.tile([128, 2 * CC], f32, tag="bqk")
            nc.sync.dma_start(bqk, bqk_d.rearrange("(m p) -> p m", p=128))
            wv = singles.tile([128, CC, C], bf16, tag="wv")
            nc.sync.dma_start(wv, wv_d.rearrange("(c p) f -> p c f", p=128))
            bv = singles.tile([128, C], f32, tag="bv")
            nc.gpsimd.dma_start(bv, bass.AP(tensor=bv_d.tensor,
                                            offset=bv_d.offset,
                                            ap=[[0, 128], [1, C]]))
            wp = singles.tile([128, CC, C], bf16, tag="wp")
            nc.sync.dma_start(wp, wp_d.rearrange("(c p) f -> p c f", p=128))
            bp = singles.tile([128, C], f32, tag="bp")
            nc.gpsimd.dma_start(bp, bass.AP(tensor=bp_d.tensor,
                                            offset=bp_d.offset,
                                            ap=[[0, 128], [1, C]]))
            w1 = singles.tile([128, CC, HID], bf16, tag="w1")
            nc.sync.dma_start(w1, w1_d.rearrange("(c p) f -> p c f", p=128))
            b1 = singles.tile([128, MH], f32, tag="b1")
            nc.sync.dma_start(b1, b1_d.rearrange("(m p) -> p m", p=128))
            w2 = singles.tile([128, MH, C], bf16, tag="w2")
            nc.sync.dma_start(w2, w2_d.rearrange("(m p) f -> p m f", p=128))
            b2 = singles.tile([128, C], f32, tag="b2")
            nc.gpsimd.dma_start(b2, bass.AP(tensor=b2_d.tensor,
                                            offset=b2_d.offset,
                                            ap=[[0, 128], [1, C]]))

            nc.vector.memset(vaug, 0.0)
            nc.vector.memset(vaug[:, :, 0:HEADS:2, 64:65], 1.0)   # even heads
            nc.vector.memset(vaug[:, :, 1:HEADS:2, 0:1], 1.0)     # odd heads

            def layer_norm(x_t, z_t):
                """z_t(bf16) = (x_t - mean)/sqrt(var+eps); gain/bias folded into W."""
                st = stats.tile([128, 6], f32, tag="bnst")
                nc.vector.bn_stats(st, x_t)
                mv = stats.tile([128, 2], f32, tag="bnmv")
                nc.vector.bn_aggr(mv, st)
                rstd = stats.tile([128, 1], f32, tag="rstd")
                nc.scalar.activation(rstd, mv[:, 1:2], AF.Sqrt, bias=eps_t,
                                     scale=1.0)
                nc.vector.reciprocal(rstd, rstd)
                nc.vector.tensor_scalar(z_t, x_t, mv[:, 0:1], rstd,
                                        op0=ALU.subtract, op1=ALU.mult)

            # ---- Phase A: load x, LN1, transpose to hT ----
            for i in range(NT):
                if i < NTQ:
                    x_t = xq[:, i, :]
                else:
                    x_t = work.tile([128, C], f32, tag="xt")
                nc.sync.dma_start(x_t, xkv_d[i * 128:(i + 1) * 128, :])
                z = work.tile([128, C], bf16, tag="z")
                layer_norm(x_t, z)
                for c in range(CC):
                    nc.sync.dma_start(hT[c][:, i * 128:(i + 1) * 128],
                                      z[:, c * 128:(c + 1) * 128],
                                      transpose=True)

            # ---- Phase B: QKV projections ----
            # Q^T / K^T feature-major: lhsT = wqk chunk, rhs = hT
            for m in range(2 * CC):          # m<3 -> Q chunks, m>=3 -> K chunks
                is_q = m < CC
                dst = qT[m] if is_q else kT[m - CC]
                ncols = TQ if is_q else T
                for n2 in range(ncols // 1024):
                    ps = psumA.tile([128, 1024], f32, tag="A")
                    for h2 in range(2):
                        n0 = n2 * 1024 + h2 * 512
                        for c in range(CC):
                            nc.tensor.matmul(
                                ps[:, h2 * 512:(h2 + 1) * 512],
                                wqk[:, c, m * 128:(m + 1) * 128],
                                hT[c][:, n0:n0 + 512],
                                start=(c == 0), stop=(c == CC - 1))
                    nc.vector.tensor_scalar_add(
                        dst[:, n2 * 1024:(n2 + 1) * 1024], ps, bqk[:, m:m + 1])
            # V token-major (with bias broadcast along features)
            for tk in range(NT):
                ps = psumB.tile([128, C], f32, tag="B")
                for c in range(CC):
                    nc.tensor.matmul(ps, hT[c][:, tk * 128:(tk + 1) * 128],
                                     wv[:, c, :], start=(c == 0),
                                     stop=(c == CC - 1))
                ps_h = ps.rearrange("p (h d) -> p h d", h=HEADS)
                bv_h = bv.rearrange("p (h d) -> p h d", h=HEADS)
                nc.vector.tensor_tensor(
                    vaug[:, tk, 0:HEADS:2, 0:HEAD_DIM],
                    ps_h[:, 0:HEADS:2, :], bv_h[:, 0:HEADS:2, :], ALU.add)
                nc.vector.tensor_tensor(
                    vaug[:, tk, 1:HEADS:2, 64:128],
                    ps_h[:, 1:HEADS:2, :], bv_h[:, 1:HEADS:2, :], ALU.add)

            # ---- Phase C: attention (per head; S^T then exp then PV) ----
            for h in range(HEADS):
                hp, off = h // 2, (h % 2) * 64
                odd = h % 2 == 1
                nv = 128 if odd else HEAD_DIM + 1
                o_ps = psumB.tile([128, TQ], f32, tag="B")
                for kc in range(NT):
                    s_ps = psumA.tile([128, TQ], f32, tag="A")
                    for qh in range(2):
                        nc.tensor.matmul(
                            s_ps[:, qh * 512:(qh + 1) * 512],
                            kT[hp][off:off + 64, kc * 128:(kc + 1) * 128],
                            qT[hp][off:off + 64, qh * 512:(qh + 1) * 512],
                            start=True, stop=True, tile_position=(off, 0))
                    a_t = attn_pool.tile([128, TQ], bf16, tag="attn")
                    nc.scalar.activation(a_t, s_ps, AF.Exp)
                    for qh in range(2):
                        nc.tensor.matmul(
                            o_ps[0:nv, qh * 512:(qh + 1) * 512],
                            vaug[:, kc, h, 0:nv],
                            a_t[:, qh * 512:(qh + 1) * 512],
                            start=(kc == 0), stop=(kc == NT - 1))
                # denominator row sits at partition 64 (even) / 63 (odd);
                # reciprocal it in place-partition, DMA-broadcast onto the
                # head's 64 partitions, then scale the O block into oT.
                dn = 64 if not odd else 0
                rec = stats.tile([128, TQ], f32, tag="rec", bufs=2)
                nc.vector.reciprocal(rec[dn:dn + 1, :], o_ps[dn:dn + 1, :])
                rec_dram = dram.tile([1, TQ], f32, tag="recd", bufs=2)
                nc.sync.dma_start(rec_dram, rec[dn:dn + 1, :])
                rec_bcast = bass.AP(tensor=rec_dram.tensor,
                                    offset=rec_dram.offset,
                                    ap=[[0, HEAD_DIM], [1, TQ]])
                nc.sync.dma_start(rec[off:off + HEAD_DIM, :], rec_bcast)
                nc.vector.tensor_tensor(oT[hp][off:off + HEAD_DIM, :],
                                        o_ps[off:off + HEAD_DIM, :],
                                        rec[off:off + HEAD_DIM, :], ALU.mult)

            # ---- Phase D: proj + residual -> x2; LN2; transpose to x2T ----
            for tq in range(NTQ):
                ps = psumB.tile([128, C], f32, tag="B")
                for c in range(CC):
                    nc.tensor.matmul(ps, oT[c][:, tq * 128:(tq + 1) * 128],
                                     wp[:, c, :], start=(c == 0),
                                     stop=(c == CC - 1))
                x2_t = x2[:, tq, :]
                nc.vector.tensor_add(x2_t, ps, xq[:, tq, :])
                nc.vector.tensor_tensor(x2_t, x2_t, bp, ALU.add)
                z2 = work.tile([128, C], bf16, tag="z")
                layer_norm(x2_t, z2)
                for c in range(CC):
                    nc.sync.dma_start(x2T[c][:, tq * 128:(tq + 1) * 128],
                                      z2[:, c * 128:(c + 1) * 128],
                                      transpose=True)

            # ---- Phase E: MLP ----
            for m in range(MH):
                ps = psumA.tile([128, TQ], f32, tag="A")
                for qh in range(2):
                    for c in range(CC):
                        nc.tensor.matmul(
                            ps[:, qh * 512:(qh + 1) * 512],
                            w1[:, c, m * 128:(m + 1) * 128],
                            x2T[c][:, qh * 512:(qh + 1) * 512],
                            start=(c == 0), stop=(c == CC - 1))
                act_fn = AF.Tanh if sim_gelu else AF.Gelu
                nc.scalar.activation(gT[:, m, :], ps, act_fn,
                                     bias=b1[:, m:m + 1], scale=1.0)
            for tq in range(NTQ):
                ps = psumB.tile([128, C], f32, tag="B")
                for m in range(MH):
                    nc.tensor.matmul(ps, gT[:, m, tq * 128:(tq + 1) * 128],
                                     w2[:, m, :], start=(m == 0),
                                     stop=(m == MH - 1))
                o_t = work.tile([128, C], f32, tag="ot")
                nc.vector.tensor_add(o_t, ps, x2[:, tq, :])
                nc.vector.tensor_tensor(o_t, o_t, b2, ALU.add)
                nc.sync.dma_start(out_d[tq * 128:(tq + 1) * 128, :], o_t)

    nc.compile()
    return nc


def prep_inputs(x, ln1_g, ln1_b, qkv_w, qkv_b, proj_w, proj_b,
                ln2_g, ln2_b, fc1_w, fc1_b, fc2_w, fc2_b):
    """Host-side folding + per-core input maps."""
    bf16 = ml_dtypes.bfloat16
    x = np.asarray(x, np.float32)
    r = float(HEAD_DIM ** -0.25)
    qkv_w = np.asarray(qkv_w, np.float32)
    w_eff = np.asarray(ln1_g, np.float32)[:, None] * qkv_w
    b_eff = np.asarray(ln1_b, np.float32) @ qkv_w + np.asarray(qkv_b, np.float32)
    wq = w_eff[:, :C] * r
    wk = w_eff[:, C:2 * C] * r
    bq = b_eff[:C] * r
    bk = b_eff[C:2 * C] * r
    wv = w_eff[:, 2 * C:]
    bv = b_eff[2 * C:]
    fc1_w = np.asarray(fc1_w, np.float32)
    w1_eff = np.asarray(ln2_g, np.float32)[:, None] * fc1_w
    b1_eff = np.asarray(ln2_b, np.float32) @ fc1_w + np.asarray(fc1_b, np.float32)

    shared = {
        "wqk": np.ascontiguousarray(np.concatenate([wq, wk], axis=1)).astype(bf16),
        "bqk": np.ascontiguousarray(np.concatenate([bq, bk])).astype(np.float32),
        "wv": np.ascontiguousarray(wv).astype(bf16),
        "bv": np.ascontiguousarray(bv).astype(np.float32),
        "wp": np.asarray(proj_w, np.float32).astype(bf16),
        "bp": np.asarray(proj_b, np.float32),
        "w1": np.ascontiguousarray(w1_eff).astype(bf16),
        "b1": np.ascontiguousarray(b1_eff).astype(np.float32),
        "w2": np.asarray(fc2_w, np.float32).astype(bf16),
        "b2": np.asarray(fc2_b, np.float32),
    }
    in_maps = []
    for c in range(NCORES):
        b, half = c // 2, c % 2
        xb = x[b]
        xkv = np.concatenate([xb[half * TQ:(half + 1) * TQ],
                              xb[(1 - half) * TQ:(2 - half) * TQ]], axis=0)
        in_maps.append({"xkv": np.ascontiguousarray(xkv), **shared})
    return in_maps


def kernel(**inputs):
    global _COMPILED
    from concourse import bass_utils

    x = np.asarray(inputs["x"], np.float32)
    assert x.shape == (B, N, C), x.shape
    in_maps = prep_inputs(**inputs)
    if _COMPILED is None:
        _COMPILED = build_nc()
    nc = _COMPILED
    res = bass_utils.run_bass_kernel_spmd(nc, in_maps,
                                          core_ids=list(range(NCORES)))
    out = np.empty((B, N, C), np.float32)
    for c in range(NCORES):
        b, half = c // 2, c % 2
        out[b, half * TQ:(half + 1) * TQ] = res.results[c]["out"]
    return out


# revision 16
# speedup vs baseline: 1.0404x; 1.0404x over previous
"""Trainium2 Bass kernel for a pre-norm transformer block (dense_transformer).

Shapes (hardcoded): x [B=4, N=2048, C=384], HEADS=6, HEAD_DIM=64, HID=1536.

Sharding: 8 cores = (batch, query-half). Core c handles batch b=c//2 and query
rows half=c%2. Each core receives its batch's full 2048 tokens, reordered so
its own 1024 query rows come first (attention keys are permutation-invariant).
It computes LN1 -> QKV (K/V for all 2048 tokens, Q for its 1024), dense
attention for all 6 heads, proj + residual, LN2, MLP + residual, and writes its
1024 output rows. No cross-core communication.

Device-side structure (v2):
  - Host supplies x both token-major (LN stats, residual) and feature-major
    xT (matmul operand), so no on-chip transposes are needed: the LN apply is
    factored as zT = xT*rstd_bcast - (mean*rstd)_bcast, with the per-token
    stats rows broadcast across partitions via a tiny DRAM bounce.
  - All matmul activations bf16 (fp32 residual stream); LN gain/bias and the
    1/sqrt(d) attention scale are folded into the weights host-side.
  - Attention per head-pair: S^T = K_h Q_h^T for two heads issued to disjoint
    PE row-groups (head_dim=64 -> concurrent), exp on ScalarE straight out of
    PSUM (no max-subtraction: |scores| is provably < ~30 so exp can't
    overflow), softmax denominator via a ones column in the padded V block,
    reciprocal via the ~2ULP approx DVE op, denominator row broadcast across
    partitions through a DRAM bounce.
  - MLP hidden activations stay feature-major end-to-end (x2T built from a
    second, feature-major proj matmul instead of a transpose).
"""

import numpy as np
import ml_dtypes

B, N, C = 4, 2048, 384
HEADS, HEAD_DIM = 6, 64
HID = 1536
EPS = 1e-5
NCORES = 8
T = N            # tokens per core (full batch element)
TQ = N // 2      # query rows per core
CC = C // 128    # 3 feature chunks
NT = T // 128    # 16 token chunks
NTQ = TQ // 128  # 8 query-token chunks
MH = HID // 128  # 12 hidden chunks

_COMPILED = None


def build_nc(sim_gelu=False):
    """Build + compile the per-core Bass/Tile program (same for all cores)."""
    import concourse.bass as bass
    import concourse.tile as tile
    from concourse import bacc, mybir

    f32 = mybir.dt.float32
    bf16 = mybir.dt.bfloat16
    AF = mybir.ActivationFunctionType
    ALU = mybir.AluOpType

    nc = bacc.Bacc("TRN2", target_bir_lowering=False, debug=False,
                   num_devices=NCORES)

    xkv_d = nc.dram_tensor("xkv", [T, C], f32, kind="ExternalInput").ap()
    xt_d = nc.dram_tensor("xt", [C, T], f32, kind="ExternalInput").ap()
    wqk_d = nc.dram_tensor("wqk", [C, 2 * C], bf16, kind="ExternalInput").ap()
    bqk_d = nc.dram_tensor("bqk", [2 * C], f32, kind="ExternalInput").ap()
    wv_d = nc.dram_tensor("wv", [C, C], bf16, kind="ExternalInput").ap()
    bv_d = nc.dram_tensor("bv", [C], f32, kind="ExternalInput").ap()
    wp_d = nc.dram_tensor("wp", [C, C], bf16, kind="ExternalInput").ap()
    bp_d = nc.dram_tensor("bp", [C], f32, kind="ExternalInput").ap()
    w1_d = nc.dram_tensor("w1", [C, HID], bf16, kind="ExternalInput").ap()
    b1_d = nc.dram_tensor("b1", [HID], f32, kind="ExternalInput").ap()
    w2_d = nc.dram_tensor("w2", [HID, C], bf16, kind="ExternalInput").ap()
    b2_d = nc.dram_tensor("b2", [C], f32, kind="ExternalInput").ap()
    out_d = nc.dram_tensor("out", [TQ, C], f32, kind="ExternalOutput").ap()

    def bcast_load(engine, dst, src_ap, parts=128):
        """DMA a DRAM row into `parts` partitions (partition-broadcast)."""
        engine.dma_start(dst, bass.AP(tensor=src_ap.tensor,
                                      offset=src_ap.offset,
                                      ap=[[0, parts]] + list(src_ap.ap)))

    with tile.TileContext(nc) as tc:
        with (
            tc.tile_pool(name="singles", bufs=1) as singles,
            tc.tile_pool(name="work", bufs=4) as work,
            tc.tile_pool(name="stats", bufs=6) as stats,
            tc.tile_pool(name="attn", bufs=3) as attn_pool,
            tc.tile_pool(name="psumA", bufs=2, space="PSUM") as psumA,
            tc.tile_pool(name="psumB", bufs=4, space="PSUM") as psumB,
            tc.tile_pool(name="dram", bufs=4, space="DRAM") as dram,
        ):
            # ---- PE warmup: dummy matmuls so the HAM clock-gate opens before
            # the real work arrives (PE is otherwise idle for ~20us). ----
            warm_w = singles.tile([128, 128], bf16, tag="warm_w")
            warm_x = singles.tile([128, 512], bf16, tag="warm_x")
            nc.vector.memset(warm_w, 0.0)
            nc.vector.memset(warm_x, 0.0)
            for wi in range(40):
                wps = psumA.tile([128, 512], f32, tag="A", name=f"warm{wi}")
                nc.tensor.matmul(wps, warm_w, warm_x, start=True, stop=True)

            # ---- persistent SBUF tensors (tags shared across phases to fit
            # SBUF: xt3->gT and qT->x2z reuse the same slots) ----
            xq = singles.tile([128, NTQ, C], f32, tag="xq")
            xt3 = singles.tile([128, CC, T], f32, tag="big24")
            zT = singles.tile([128, CC, T], bf16, tag="zT")
            qT = singles.tile([128, CC, TQ], bf16, tag="qx")
            kT = singles.tile([128, CC, T], bf16, tag="kT")
            vauge = singles.tile([128, NT, 3, HEAD_DIM + 1], bf16, tag="vauge")
            vaugo = singles.tile([128, NT, 3, 128], bf16, tag="vaugo")
            oT = singles.tile([128, CC, TQ], bf16, tag="oT")
            x2 = singles.tile([128, NTQ, C], f32, tag="x2")
            eps_t = singles.tile([128, 1], f32, tag="eps")
            nc.vector.memset(eps_t, EPS)
            st_s = singles.tile([128, NT], f32, tag="st_s")
            st_b = singles.tile([128, NT], f32, tag="st_b")
            st2_s = singles.tile([128, NTQ], f32, tag="st2_s")
            st2_b = singles.tile([128, NTQ], f32, tag="st2_b")

            # odd-head V layout [ones(0) | zeros(1:64) | V(64:128)]
            nc.gpsimd.memset(vaugo[:, :, :, 0:HEAD_DIM], 0.0)
            nc.gpsimd.memset(vaugo[:, :, :, 0:1], 1.0)
            nc.gpsimd.memset(vauge[:, :, :, HEAD_DIM:HEAD_DIM + 1], 1.0)

            # ---- weights ----
            wqk = singles.tile([128, CC, 2 * C], bf16, tag="wqk")
            nc.sync.dma_start(wqk, wqk_d.rearrange("(c p) f -> p c f", p=128))
            bqk = singles.tile([128, 2 * CC], f32, tag="bqk")
            nc.sync.dma_start(bqk, bqk_d.rearrange("(m p) -> p m", p=128))
            wv = singles.tile([128, CC, C], bf16, tag="wv")
            nc.sync.dma_start(wv, wv_d.rearrange("(c p) f -> p c f", p=128))
            bvB = singles.tile([128, C], f32, tag="bvB")
            bcast_load(nc.gpsimd, bvB, bv_d)
            wp = singles.tile([128, CC, C], bf16, tag="wp")
            nc.sync.dma_start(wp, wp_d.rearrange("(c p) f -> p c f", p=128))
            bpB = singles.tile([128, C], f32, tag="bpB")
            bcast_load(nc.gpsimd, bpB, bp_d)
            bpT = singles.tile([128, CC], f32, tag="bpT")
            nc.sync.dma_start(bpT, bp_d.rearrange("(c p) -> p c", p=128))
            w1 = singles.tile([128, CC, HID], bf16, tag="w1")
            nc.sync.dma_start(w1, w1_d.rearrange("(c p) f -> p c f", p=128))
            b1c = singles.tile([128, MH], f32, tag="b1c")
            nc.sync.dma_start(b1c, b1_d.rearrange("(m p) -> p m", p=128))
            w2 = singles.tile([128, MH, C], bf16, tag="w2")
            nc.sync.dma_start(w2, w2_d.rearrange("(m p) f -> p m f", p=128))
            b2B = singles.tile([128, C], f32, tag="b2B")
            bcast_load(nc.gpsimd, b2B, b2_d)

            # ---- x loads ----
            nc.sync.dma_start(xq,
                              xkv_d[0:TQ].rearrange("(i p) f -> p i f", p=128))
            nc.sync.dma_start(xt3, xt_d.rearrange("(c p) t -> p c t", p=128))

            def ln_stats(x_t, s_col, b_col):
                """s_col = 1/sqrt(var+eps); b_col = mean*s  (per-token)."""
                st = stats.tile([128, 6], f32, tag="bnst")
                nc.vector.bn_stats(st, x_t)
                mv = stats.tile([128, 2], f32, tag="bnmv")
                nc.vector.bn_aggr(mv, st)
                nc.scalar.activation(s_col, mv[:, 1:2], AF.Sqrt, bias=eps_t,
                                     scale=1.0)
                nc.vector.reciprocal(s_col, s_col)
                nc.vector.tensor_tensor(b_col, mv[:, 0:1], s_col, ALU.mult)

            # ---- Phase A: LN1 stats over all 16 token chunks ----
            for i in range(NT):
                if i < NTQ:
                    x_t = xq[:, i, :]
                else:
                    x_t = work.tile([128, C], f32, tag="xkvt")
                    nc.sync.dma_start(x_t, xkv_d[i * 128:(i + 1) * 128, :])
                ln_stats(x_t, st_s[:, i:i + 1], st_b[:, i:i + 1])

            # bounce stats rows to DRAM, reload partition-broadcast
            s_dram = dram.tile([T], f32, tag="s_dram")
            b_dram = dram.tile([T], f32, tag="b_dram")
            nc.sync.dma_start(s_dram.rearrange("(i p) -> p i", p=128), st_s)
            nc.sync.dma_start(b_dram.rearrange("(i p) -> p i", p=128), st_b)
            sB = singles.tile([128, T], f32, tag="bc0")
            bB = singles.tile([128, T], f32, tag="bc1")
            bcast_load(nc.sync, sB, s_dram)
            bcast_load(nc.sync, bB, b_dram)

            # zT = xT*sB - bB  (bf16), in 512-wide slices
            for c in range(CC):
                for s in range(T // 512):
                    sl = slice(s * 512, (s + 1) * 512)
                    t1 = work.tile([128, 512], f32, tag="zf")
                    nc.vector.tensor_tensor(t1, xt3[:, c, sl], sB[:, sl],
                                            ALU.mult)
                    nc.vector.tensor_tensor(zT[:, c, sl], t1, bB[:, sl],
                                            ALU.subtract)

            # ---- Phase B: QKV ----
            for m in range(2 * CC):          # m<3 -> Q chunks, else K chunks
                is_q = m < CC
                ncols = TQ if is_q else T
                for n2 in range(ncols // 1024):
                    ps = psumA.tile([128, 1024], f32, tag="A")
                    for h2 in range(2):
                        n0 = n2 * 1024 + h2 * 512
                        for c in range(CC):
                            nc.tensor.matmul(
                                ps[:, h2 * 512:(h2 + 1) * 512],
                                wqk[:, c, m * 128:(m + 1) * 128],
                                zT[:, c, n0:n0 + 512],
                                start=(c == 0), stop=(c == CC - 1))
                    dst = (qT[:, m, :] if is_q else
                           kT[:, m - CC, n2 * 1024:(n2 + 1) * 1024])
                    nc.vector.tensor_scalar_add(dst, ps, bqk[:, m:m + 1])
            for tk in range(NT):
                ps = psumB.tile([128, C], f32, tag="B")
                for c in range(CC):
                    nc.tensor.matmul(ps, zT[:, c, tk * 128:(tk + 1) * 128],
                                     wv[:, c, :], start=(c == 0),
                                     stop=(c == CC - 1))
                ps_h = ps.rearrange("p (h d) -> p h d", h=HEADS)
                bv_h = bvB.rearrange("p (h d) -> p h d", h=HEADS)
                nc.vector.tensor_tensor(
                    vauge[:, tk, :, 0:HEAD_DIM],
                    ps_h[:, 0:HEADS:2, :], bv_h[:, 0:HEADS:2, :], ALU.add)
                nc.vector.tensor_tensor(
                    vaugo[:, tk, :, HEAD_DIM:128],
                    ps_h[:, 1:HEADS:2, :], bv_h[:, 1:HEADS:2, :], ALU.add)

            # ---- Phase C: attention, one head-pair x query-half at a time --
            for hp in range(CC):
                for qh in range(2):
                    qsl = slice(qh * 512, (qh + 1) * 512)
                    o_e = psumB.tile([128, 512], f32, tag="B",
                                     name=f"oe{hp}{qh}")
                    o_o = psumB.tile([128, 512], f32, tag="B",
                                     name=f"oo{hp}{qh}")
                    for kc in range(NT):
                        s_ps = psumA.tile([128, 1024], f32, tag="A")
                        ksl = slice(kc * 128, (kc + 1) * 128)
                        nc.tensor.matmul(s_ps[:, 0:512],
                                         kT[0:64, hp, ksl],
                                         qT[0:64, hp, qsl],
                                         start=True, stop=True,
                                         tile_position=(0, 0))
                        nc.tensor.matmul(s_ps[:, 512:1024],
                                         kT[64:128, hp, ksl],
                                         qT[64:128, hp, qsl],
                                         start=True, stop=True,
                                         tile_position=(64, 0))
                        a_t = attn_pool.tile([128, 1024], bf16, tag="attn")
                        nc.scalar.activation(a_t, s_ps, AF.Exp)
                        nc.tensor.matmul(o_e[0:HEAD_DIM + 1, :],
                                         vauge[:, kc, hp, :], a_t[:, 0:512],
                                         start=(kc == 0), stop=(kc == NT - 1))
                        nc.tensor.matmul(o_o, vaugo[:, kc, hp, :],
                                         a_t[:, 512:1024],
                                         start=(kc == 0), stop=(kc == NT - 1))
                    # normalize: denominator row -> approx-reciprocal ->
                    # partition-broadcast via DRAM bounce -> scale into oT
                    for parity, o_ps in ((0, o_e), (1, o_o)):
                        dn = HEAD_DIM if parity == 0 else 0
                        off = 0 if parity == 0 else 64
                        rec = stats.tile([128, 512], f32, tag="rec", bufs=2)
                        nc.vector.reciprocal(rec[dn:dn + 1, :],
                                             o_ps[dn:dn + 1, :])
                        r_dram = dram.tile([512], f32, tag="r_dram", bufs=4)
                        nc.sync.dma_start(r_dram[None, :], rec[dn:dn + 1, :])
                        bcast_load(nc.sync, rec[off:off + HEAD_DIM, :],
                                   r_dram, parts=HEAD_DIM)
                        nc.vector.tensor_tensor(
                            oT[off:off + HEAD_DIM, hp, qsl],
                            o_ps[off:off + HEAD_DIM, :],
                            rec[off:off + HEAD_DIM, :], ALU.mult)

            # ---- Phase D: proj + residual -> x2 (token-major) + LN2;
            #      feature-major proj -> x2z ----
            for tq in range(NTQ):
                ps = psumB.tile([128, C], f32, tag="B")
                for c in range(CC):
                    nc.tensor.matmul(ps, oT[:, c, tq * 128:(tq + 1) * 128],
                                     wp[:, c, :], start=(c == 0),
                                     stop=(c == CC - 1))
                x2_t = x2[:, tq, :]
                nc.vector.tensor_add(x2_t, ps, xq[:, tq, :])
                nc.vector.tensor_tensor(x2_t, x2_t, bpB, ALU.add)
                ln_stats(x2_t, st2_s[:, tq:tq + 1], st2_b[:, tq:tq + 1])
            s2_dram = dram.tile([TQ], f32, tag="s2_dram")
            b2_dram = dram.tile([TQ], f32, tag="b2_dram")
            nc.sync.dma_start(s2_dram.rearrange("(i p) -> p i", p=128), st2_s)
            nc.sync.dma_start(b2_dram.rearrange("(i p) -> p i", p=128), st2_b)
            s2B = singles.tile([128, TQ], f32, tag="bc0", name="s2B")
            b2Bt = singles.tile([128, TQ], f32, tag="bc1", name="b2Bt")
            bcast_load(nc.sync, s2B, s2_dram)
            bcast_load(nc.sync, b2Bt, b2_dram)

            x2z = singles.tile([128, CC, TQ], bf16, tag="qx", name="x2z")
            for c in range(CC):
                ps = psumA.tile([128, 1024], f32, tag="A")
                for qh in range(2):
                    for kc in range(CC):
                        nc.tensor.matmul(
                            ps[:, qh * 512:(qh + 1) * 512],
                            wp[:, kc, c * 128:(c + 1) * 128],
                            oT[:, kc, qh * 512:(qh + 1) * 512],
                            start=(kc == 0), stop=(kc == CC - 1))
                x2t_f = work.tile([128, TQ], f32, tag="x2tf", bufs=2)
                nc.vector.tensor_add(x2t_f, ps, xt3[:, c, 0:TQ])
                nc.vector.tensor_scalar_add(x2t_f, x2t_f, bpT[:, c:c + 1])
                nc.vector.tensor_tensor(x2t_f, x2t_f, s2B, ALU.mult)
                nc.vector.tensor_tensor(x2z[:, c, :], x2t_f, b2Bt,
                                        ALU.subtract)

            # ---- Phase E: MLP ----
            gT = singles.tile([128, MH, TQ], bf16, tag="big24", name="gT")
            act_fn = AF.Tanh if sim_gelu else AF.Gelu
            for m in range(MH):
                ps = psumA.tile([128, 1024], f32, tag="A")
                for qh in range(2):
                    for c in range(CC):
                        nc.tensor.matmul(
                            ps[:, qh * 512:(qh + 1) * 512],
                            w1[:, c, m * 128:(m + 1) * 128],
                            x2z[:, c, qh * 512:(qh + 1) * 512],
                            start=(c == 0), stop=(c == CC - 1))
                nc.scalar.activation(gT[:, m, :], ps, act_fn,
                                     bias=b1c[:, m:m + 1], scale=1.0)
            for tq in range(NTQ):
                ps = psumB.tile([128, C], f32, tag="B")
                for m in range(MH):
                    nc.tensor.matmul(ps, gT[:, m, tq * 128:(tq + 1) * 128],
                                     w2[:, m, :], start=(m == 0),
                                     stop=(m == MH - 1))
                o_t = work.tile([128, C], f32, tag="ot")
                nc.vector.tensor_add(o_t, ps, x2[:, tq, :])
                nc.vector.tensor_tensor(o_t, o_t, b2B, ALU.add)
                nc.sync.dma_start(out_d[tq * 128:(tq + 1) * 128, :], o_t)

    nc.compile()
    return nc


def prep_inputs(x, ln1_g, ln1_b, qkv_w, qkv_b, proj_w, proj_b,
                ln2_g, ln2_b, fc1_w, fc1_b, fc2_w, fc2_b):
    """Host-side folding + per-core input maps."""
    bf16 = ml_dtypes.bfloat16
    x = np.asarray(x, np.float32)
    r = float(HEAD_DIM ** -0.25)
    qkv_w = np.asarray(qkv_w, np.float32)
    w_eff = np.asarray(ln1_g, np.float32)[:, None] * qkv_w
    b_eff = np.asarray(ln1_b, np.float32) @ qkv_w + np.asarray(qkv_b, np.float32)
    wq = w_eff[:, :C] * r
    wk = w_eff[:, C:2 * C] * r
    bq = b_eff[:C] * r
    bk = b_eff[C:2 * C] * r
    wv = w_eff[:, 2 * C:]
    bv = b_eff[2 * C:]
    fc1_w = np.asarray(fc1_w, np.float32)
    w1_eff = np.asarray(ln2_g, np.float32)[:, None] * fc1_w
    b1_eff = np.asarray(ln2_b, np.float32) @ fc1_w + np.asarray(fc1_b, np.float32)

    shared = {
        "wqk": np.ascontiguousarray(np.concatenate([wq, wk], axis=1)).astype(bf16),
        "bqk": np.ascontiguousarray(np.concatenate([bq, bk])).astype(np.float32),
        "wv": np.ascontiguousarray(wv).astype(bf16),
        "bv": np.ascontiguousarray(bv).astype(np.float32),
        "wp": np.asarray(proj_w, np.float32).astype(bf16),
        "bp": np.asarray(proj_b, np.float32),
        "w1": np.ascontiguousarray(w1_eff).astype(bf16),
        "b1": np.ascontiguousarray(b1_eff).astype(np.float32),
        "w2": np.asarray(fc2_w, np.float32).astype(bf16),
        "b2": np.asarray(fc2_b, np.float32),
    }
    in_maps = []
    for c in range(NCORES):
        b, half = c // 2, c % 2
        xb = x[b]
        xkv = np.concatenate([xb[half * TQ:(half + 1) * TQ],
                              xb[(1 - half) * TQ:(2 - half) * TQ]], axis=0)
        in_maps.append({"xkv": np.ascontiguousarray(xkv),
                        "xt": np.ascontiguousarray(xkv.T), **shared})
    return in_maps


def kernel(**inputs):
    global _COMPILED
    from concourse import bass_utils

    x = np.asarray(inputs["x"], np.float32)
    assert x.shape == (B, N, C), x.shape
    in_maps = prep_inputs(**inputs)
    if _COMPILED is None:
        _COMPILED = build_nc()
    nc = _COMPILED
    res = bass_utils.run_bass_kernel_spmd(nc, in_maps,
                                          core_ids=list(range(NCORES)))
    out = np.empty((B, N, C), np.float32)
    for c in range(NCORES):
        b, half = c // 2, c % 2
        out[b, half * TQ:(half + 1) * TQ] = res.results[c]["out"]
    return out
